# revision 1
# baseline (speedup 1.0000x reference)
"""Trainium2 Bass kernel for AdvancedNeuralMemory (B=4, S=8192, D=1024, M=512).

Math notes
----------
The recurrence  s_t = g * s_{t-1} + u_t  has a *scalar constant* gate
g = sigmoid(forget_factor) ~= 0.525, so  mem_t = sum_{j<=t} g^(t-j) u_j.
g^129 ~ 7e-37, far below fp32 resolution, so a 256-step window is exact in
fp32: for 128-row time tiles,
    mem_tile_i = Tprev.T @ u_{i-1} + Tcur.T @ u_i
with host-precomputed decay-Toeplitz matrices (adaptive_lr folded in).
This removes the sequential dependency entirely -> pure matmuls.

Sharding: 8 cores = (batch 0..3) x (seq half 0..1). Each core processes a
[4096, 1024] slab plus a 128-row halo tile (for u_{i-1} of its first tile).
No cross-device communication.

Layout: sequence-on-partitions ([128 s-rows, features] tiles); matmul
contractions get their lhsT via PE transposes. All big matmuls run as
float32r (full fp32 data, 1 cycle/row when free dim >= 256).
"""

import sys
import os

for _p in ("/opt/trn_rl_repo",):
    if _p not in sys.path and os.path.isdir(_p):
        sys.path.insert(0, _p)

from contextlib import ExitStack

import numpy as np

import concourse.bass as bass
import concourse.mybir as mybir
import concourse.tile as tile
from concourse.bass_utils import run_bass_kernel_spmd

B, S, D, M = 4, 8192, 1024, 512
HALF = S // 2          # rows per core
TS = 128               # s-tile rows
NT = HALF // TS        # compute tiles per core (32)
SLAB = HALF + TS       # slab rows incl. halo tile
LN_EPS = 1e-5
N_CORES = 8
# packed weights: wd(8*512) wq/wk/wv/w1/w2(4*512 each) wu(4*1024) tt(2*128)
WPACK_COLS = 8 * M + 5 * 4 * M + 4 * D + 2 * TS

f32 = mybir.dt.float32
f32r = mybir.dt.float32r
AF = mybir.ActivationFunctionType
ALU = mybir.AluOpType

# test.py can flip these
TRACE = False
TRACE_KWARGS = {}
LAST_RESULTS = None    # BassKernelResults of the last run (exec_time_ns etc.)

_PROG_CACHE = {}


def _r(ap):
    """View an fp32 AP as float32r for full-rate PE matmuls."""
    return ap.bitcast(f32r)


def _fix_matmult_waits(nc):
    """Walrus allows only one sync-wait on a (fused-ldweights) Matmult.
    Move surplus waits onto an inserted NoOp on the same engine."""
    n = 0
    for f in nc.m.functions:
        for bb in f.blocks:
            insts = bb.instructions
            i = 0
            while i < len(insts):
                inst = insts[i]
                si = inst.sync_info
                tname = type(inst).__name__
                exempt = tname in ("InstNoOp",
                                   "InstEventSemaphore",
                                   "InstUnconditionalBranch", "InstCall",
                                   "InstISA", "InstRegisterMove")
                if (not exempt and si is not None and si.on_wait
                        and len(si.on_wait) > 1):
                    for w in list(si.on_wait[:-1]):
                        nop = mybir.InstNoOp(
                            name=f"wfix-{n}", ins=[], outs=[],
                            engine=inst.engine,
                            sync_info=mybir.SyncInfo(on_wait=[w],
                                                     on_update=[]))
                        insts.insert(i, nop)
                        n += 1
                        i += 1
                    si.on_wait = [si.on_wait[-1]]
                i += 1
    return n


def _build_program(flags):
    (has_bd, has_bq, has_bk, has_bv, has_gq, has_bqln, has_gk, has_bkln,
     has_bu) = flags
    nc = bass.Bass()

    x_slab = nc.declare_dram_parameter("x_slab", [SLAB, D], f32, isOutput=False)
    wpack = nc.declare_dram_parameter("wpack", [TS, WPACK_COLS], f32r,
                                      isOutput=False)
    ident = nc.declare_dram_parameter("ident", [TS, TS], f32, isOutput=False)
    hmask = nc.declare_dram_parameter("hmask", [TS, 1], f32, isOutput=False)
    opt = {}
    for name, used, shape in (
        ("bd_b", has_bd, [TS, M]), ("bq_b", has_bq, [TS, M]),
        ("bk_b", has_bk, [TS, M]), ("bv_b", has_bv, [TS, M]),
        ("gq_b", has_gq, [TS, M]), ("bqln_b", has_bqln, [TS, M]),
        ("gk_b", has_gk, [TS, M]), ("bkln_b", has_bkln, [TS, M]),
        ("bu_b", has_bu, [TS, D]),
    ):
        if used:
            opt[name] = nc.declare_dram_parameter(name, shape, f32,
                                                  isOutput=False)
    y = nc.declare_dram_parameter("y", [HALF, D], f32, isOutput=True)

    with tile.TileContext(nc) as tc, ExitStack() as ctx:
        wpool = ctx.enter_context(tc.tile_pool(name="weights", bufs=1))

        wp_sb = wpool.tile([TS, WPACK_COLS], f32r)
        nc.sync.dma_start(wp_sb[:], wpack[:])
        _off = [0]

        def _wseg(nk, ncols):
            a = _off[0]
            _off[0] += nk * ncols
            return wp_sb[:, a:_off[0]].rearrange("p (k m) -> p k m", k=nk)

        wd_sb = _wseg(8, M)
        wq_sb = _wseg(4, M)
        wk_sb = _wseg(4, M)
        wv_sb = _wseg(4, M)
        w1_sb = _wseg(4, M)
        w2_sb = _wseg(4, M)
        wu_sb = _wseg(4, D)
        tt_sb = _wseg(2, TS)
        id_sb = wpool.tile([TS, TS], f32)
        nc.sync.dma_start(id_sb[:], ident[:])
        hm_sb = wpool.tile([TS, 1], f32)
        nc.sync.dma_start(hm_sb[:], hmask[:])
        eps_sb = wpool.tile([TS, 1], f32)
        nc.vector.memset(eps_sb[:], LN_EPS)
        opt_sb = {}
        for name, h in opt.items():
            t = wpool.tile([TS, h.shape[1]], f32, tag=name, name=name)
            nc.sync.dma_start(t[:], h[:])
            opt_sb[name] = t

        # SBUF activation pools
        p_x = ctx.enter_context(tc.tile_pool(name="x", bufs=3))
        p_y = ctx.enter_context(tc.tile_pool(name="y", bufs=3))
        p_xT = ctx.enter_context(tc.tile_pool(name="xT", bufs=2))
        p_tT = ctx.enter_context(tc.tile_pool(name="tT", bufs=2))
        p_act = ctx.enter_context(tc.tile_pool(name="act", bufs=3))
        p_u = ctx.enter_context(tc.tile_pool(name="u", bufs=3))
        p_sm = ctx.enter_context(tc.tile_pool(name="sm", bufs=6))
        # PSUM pools (8 banks total: 3 + 2 + 2 = 7 used)
        p_mm = ctx.enter_context(tc.tile_pool(name="mm", bufs=4, space="PSUM"))
        p_pt = ctx.enter_context(tc.tile_pool(name="pt", bufs=2, space="PSUM"))
        p_out = ctx.enter_context(
            tc.tile_pool(name="out", bufs=1, space="PSUM"))

        def pe_transpose(src_sb, nblk, tag):
            """Transpose nblk [128,128] blocks of src_sb into a fresh SBUF
            tile laid out [128, nblk*128] (feature-on-partition)."""
            pool = p_tT if nblk <= 4 else p_xT
            dst = pool.tile([TS, nblk * TS], f32r, tag=tag, name=tag)
            for g0 in range(0, nblk, 4):
                gn = min(4, nblk - g0)
                ps = p_pt.tile([TS, 4 * TS], f32, name='ps_t', tag='ps_t')
                for j in range(gn):
                    blk = slice((g0 + j) * TS, (g0 + j + 1) * TS)
                    nc.tensor.transpose(ps[:, j * TS:(j + 1) * TS],
                                        src_sb[:, blk], id_sb[:])
                for j in range(gn):
                    dst_sl = dst[:, (g0 + j) * TS:(g0 + j + 1) * TS]
                    src_sl = ps[:, j * TS:(j + 1) * TS]
                    nc.scalar.copy(dst_sl, src_sl)
            return dst

        def layernorm(z_ps, gb, bb, tag):
            """LN over free dim of z_ps [128, M] (PSUM) -> SBUF tile."""
            st = p_sm.tile([TS, 6], f32, tag="bnst")
            nc.vector.bn_stats(st[:], z_ps[:])
            ag = p_sm.tile([TS, 2], f32, tag="bnag")
            nc.vector.bn_aggr(ag[:], st[:])
            std = p_sm.tile([TS, 1], f32, tag="std")
            nc.scalar.activation(std[:], ag[:, 1:2], AF.Sqrt, bias=eps_sb[:])
            rs = p_sm.tile([TS, 1], f32, tag="rs")
            nc.vector.reciprocal(rs[:], std[:])
            nmr = p_sm.tile([TS, 1], f32, tag="nmr")
            nc.vector.scalar_tensor_tensor(nmr[:], ag[:, 0:1], -1.0, rs[:],
                                           ALU.mult, ALU.mult)
            o = p_act.tile([TS, M], f32, tag=tag, name=tag)
            nc.scalar.activation(o[:], z_ps[:], AF.Identity,
                                 bias=nmr[:], scale=rs[:])
            if gb is not None:
                nc.vector.tensor_mul(o[:], o[:], gb[:])
            if bb is not None:
                nc.vector.tensor_add(o[:], o[:], bb[:])
            return o

        def mm_acc(out_ps, lhsT_sb, rhs_sb_3d, nk, ncols=M):
            for k in range(nk):
                nc.tensor.matmul(
                    out_ps[:, 0:ncols],
                    lhsT_sb[:, k * TS:(k + 1) * TS],
                    rhs_sb_3d[:, k, 0:ncols],
                    start=(k == 0), stop=(k == nk - 1))

        u_prev = None
        for i in range(NT + 1):
            halo = (i == 0)
            s0 = i * TS

            xt = p_x.tile([TS, D], f32)
            nc.sync.dma_start(xt[:], x_slab[s0:s0 + TS, :])

            xT = pe_transpose(xt, 8, "xT")

            h_ps = p_mm.tile([TS, M], f32, tag="mm", name="h_ps")
            mm_acc(h_ps, xT, wd_sb, 8)
            h_sb = p_act.tile([TS, M], f32, tag="h")
            nc.scalar.copy(h_sb[:], h_ps[:])
            if has_bd:
                nc.vector.tensor_add(h_sb[:], h_sb[:], opt_sb["bd_b"][:])
            hT = pe_transpose(h_sb, 4, "hT")

            if not halo:
                zq_ps = p_mm.tile([TS, M], f32, tag="mm", name="zq_ps")
                mm_acc(zq_ps, hT, wq_sb, 4)
                if has_bq:
                    nc.vector.tensor_add(zq_ps[:], zq_ps[:],
                                         opt_sb["bq_b"][:])
                q_sb = layernorm(zq_ps,
                                 opt_sb.get("gq_b"), opt_sb.get("bqln_b"),
                                 "q")

            zk_ps = p_mm.tile([TS, M], f32, tag="mm", name="zk_ps")
            mm_acc(zk_ps, hT, wk_sb, 4)
            if has_bk:
                nc.vector.tensor_add(zk_ps[:], zk_ps[:], opt_sb["bk_b"][:])
            k_sb = layernorm(zk_ps, opt_sb.get("gk_b"), opt_sb.get("bkln_b"),
                             "k")

            zv_ps = p_mm.tile([TS, M], f32, tag="mm", name="zv_ps")
            mm_acc(zv_ps, hT, wv_sb, 4)
            v_sb = p_act.tile([TS, M], f32, tag="v")
            nc.scalar.copy(v_sb[:], zv_ps[:])
            if has_bv:
                nc.vector.tensor_add(v_sb[:], v_sb[:], opt_sb["bv_b"][:])

            kT = pe_transpose(k_sb, 4, "kT")
            a1_ps = p_mm.tile([TS, M], f32, tag="mm", name="a1_ps")
            mm_acc(a1_ps, kT, w1_sb, 4)
            a1_sb = p_act.tile([TS, M], f32, tag="a1")
            nc.scalar.activation(a1_sb[:], a1_ps[:], AF.Gelu_apprx_tanh)
            a1T = pe_transpose(a1_sb, 4, "a1T")
            pred_ps = p_mm.tile([TS, M], f32, tag="mm", name="pred_ps")
            mm_acc(pred_ps, a1T, w2_sb, 4)

            u_sb = p_u.tile([TS, M], f32r)
            nc.vector.tensor_sub(u_sb[:], v_sb[:], pred_ps[:])
            if halo:
                nc.vector.tensor_scalar_mul(u_sb[:], u_sb[:], hm_sb[:, 0:1])
                u_prev = u_sb
                continue

            mem_ps = p_mm.tile([TS, M], f32, tag="mm", name="mem_ps")
            nc.tensor.matmul(mem_ps[:], tt_sb[:, 0, :], u_prev[:],
                             start=True, stop=False)
            nc.tensor.matmul(mem_ps[:], tt_sb[:, 1, :], u_sb[:],
                             start=False, stop=True)
            u_prev = u_sb

            rtr_sb = p_act.tile([TS, M], f32, tag="rtr")
            nc.vector.tensor_mul(rtr_sb[:], q_sb[:], mem_ps[:])
            rT = pe_transpose(rtr_sb, 4, "rT")

            out_ps = p_out.tile([TS, D], f32)
            for nb in range(2):
                cols = slice(nb * 512, (nb + 1) * 512)
                for k in range(4):
                    nc.tensor.matmul(
                        out_ps[:, cols],
                        rT[:, k * TS:(k + 1) * TS],
                        wu_sb[:, k, cols],
                        start=(k == 0), stop=(k == 3))

            y_sb = p_y.tile([TS, D], f32)
            nc.vector.tensor_add(y_sb[:], xt[:], out_ps[:])
            if has_bu:
                nc.vector.tensor_add(y_sb[:], y_sb[:], opt_sb["bu_b"][:])
            nc.sync.dma_start(y[s0 - TS:s0, :], y_sb[:])

    _fix_matmult_waits(nc)
    return nc


def _prep_inputs(x, Wd, bd, Wq, bq, Wk, bk, Wv, bv, gq, bq_ln, gk, bk_ln,
                 W1, W2, Wu, bu, adaptive_lr, forget_factor):
    """Host-side: flags, decay matrix, per-core slabs."""
    f = np.float32
    bd, bq, bk, bv, bu = (np.asarray(a, f) for a in (bd, bq, bk, bv, bu))
    gq, bq_ln, gk, bk_ln = (np.asarray(a, f) for a in (gq, bq_ln, gk, bk_ln))
    flags = (bool(bd.any()), bool(bq.any()), bool(bk.any()), bool(bv.any()),
             bool((gq != 1).any()), bool(bq_ln.any()),
             bool((gk != 1).any()), bool(bk_ln.any()), bool(bu.any()))

    g = 1.0 / (1.0 + np.exp(-np.float64(forget_factor)))
    lr = np.float64(adaptive_lr)
    t_idx = np.arange(TS)
    # current-tile block: coeff for u_cur[j] at output t: g^(t-j), j <= t
    lag_cur = t_idx[:, None] - t_idx[None, :]
    Tcur = np.where(lag_cur >= 0, g ** np.maximum(lag_cur, 0), 0.0) * lr
    # previous-tile block: coeff for u_prev[j]: g^(t+128-j)
    lag_prev = t_idx[:, None] + TS - t_idx[None, :]
    Tprev = (g ** lag_prev) * lr
    TT = np.concatenate([Tprev, Tcur], axis=1).T.astype(f)  # [256, 128]
    TT = np.ascontiguousarray(TT)

    def seg(w):
        w = np.asarray(w, f)          # [K, N] -> [128, nk*N]
        nk = w.shape[0] // TS
        return w.reshape(nk, TS, w.shape[1]).transpose(1, 0, 2).reshape(TS, -1)

    wpack = np.ascontiguousarray(np.concatenate(
        [seg(w) for w in (Wd, Wq, Wk, Wv, W1, W2, Wu, TT)], axis=1))
    common = {
        "wpack": wpack,
        "ident": np.eye(TS, dtype=f),
    }
    names = ("bd_b", "bq_b", "bk_b", "bv_b", "gq_b", "bqln_b", "gk_b",
             "bkln_b", "bu_b")
    vecs = (bd, bq, bk, bv, gq, bq_ln, gk, bk_ln, bu)
    for name, used, vec in zip(names, flags, vecs):
        if used:
            common[name] = np.ascontiguousarray(
                np.broadcast_to(vec, (TS, vec.shape[0])), f)

    x = np.asarray(x, f)
    in_maps = []
    for c in range(N_CORES):
        b, sh = c // 2, c % 2
        if sh == 0:
            haloblk = np.zeros((TS, D), f)
            hm = np.zeros((TS, 1), f)
        else:
            haloblk = x[b, HALF - TS:HALF]
            hm = np.ones((TS, 1), f)
        slab = np.concatenate([haloblk, x[b, sh * HALF:(sh + 1) * HALF]],
                              axis=0)
        m = dict(common)
        m["x_slab"] = np.ascontiguousarray(slab)
        m["hmask"] = hm
        in_maps.append(m)
    return flags, in_maps


def kernel(**inputs):
    global LAST_RESULTS
    flags, in_maps = _prep_inputs(**inputs)
    if flags not in _PROG_CACHE:
        _PROG_CACHE[flags] = _build_program(flags)
    nc = _PROG_CACHE[flags]

    res = run_bass_kernel_spmd(nc, in_maps, list(range(N_CORES)),
                               trace=TRACE, trace_kwargs=TRACE_KWARGS)
    LAST_RESULTS = res

    x = np.asarray(inputs["x"], np.float32)
    out = np.empty((B, S, D), np.float32)
    for c in range(N_CORES):
        b, sh = c // 2, c % 2
        out[b, sh * HALF:(sh + 1) * HALF] = res.results[c]["y"]
    return out


if __name__ == "__main__":
    rng = np.random.default_rng(0)
    print("smoke test with random inputs (not the reference distribution)")



# revision 10
# speedup vs baseline: 1.0259x; 1.0259x over previous
"""Trainium2 Bass kernel for AdvancedNeuralMemory (B=4, S=8192, D=1024, M=512).

Math notes
----------
The recurrence  s_t = g * s_{t-1} + u_t  has a *scalar constant* gate
g = sigmoid(forget_factor) ~= 0.525, so  mem_t = sum_{j<=t} g^(t-j) u_j.
g^129 ~ 7e-37, far below fp32 resolution, so a 256-step window is exact:
for 128-row time tiles,
    mem_tile_i = Tprev.T @ u_{i-1} + Tcur.T @ u_i
with host-precomputed decay-Toeplitz matrices (adaptive_lr folded in).
This removes the sequential dependency entirely -> pure matmuls.

Sharding: 8 cores = (batch 0..3) x (seq half 0..1). Each core processes a
[4096, 1024] slab plus a 128-row halo tile (for u_{i-1} of its first tile).
No cross-device communication.

v2 performance design (vs the fp32r baseline):
- all-bf16 datapath (x staged to HBM as bf16; weights bf16; y stored bf16,
  upcast on host).  bf16 matmuls run 1 col/cycle with FWL weight loads.
- no PE transposes: xT comes straight from HBM through the DMA crossbar
  transpose (dma_start_transpose), kT/rT via SBUF->SBUF crossbar DMA.
- transposed-space GEMMs avoid intermediate transposes entirely:
  hT = Wd.T @ xT (32 mm), a1T = W1.T @ kT (16 mm); row-space GEMMs
  (k/v/q from stationary hT, pred from stationary a1T, out from rT).
- the x residual is folded into the out-GEMM PSUM accumulation via an
  identity-stationary matmul (y = x + r @ Wu in one PSUM group).
- LN rsqrt on DVE (bit-trick + 2 Newton steps): the Scalar engine then only
  ever uses Identity/Copy/Gelu_apprx_tanh, which share one activation
  table -> no ACT_TABLE_LOAD swaps.
- 4-stage software pipeline: per loop iteration emits dma(t), s0(t-1),
  s1(t-2), s2(t-3), s3(t-4), so every cross-engine roundtrip has a full
  iteration of PE work in front of it.
"""

import sys
import os

for _p in ("/opt/trn_rl_repo",):
    if _p not in sys.path and os.path.isdir(_p):
        sys.path.insert(0, _p)

from contextlib import ExitStack

import numpy as np
import ml_dtypes

import concourse.bass as bass
import concourse.mybir as mybir
import concourse.tile as tile
from concourse.bass_utils import run_bass_kernel_spmd

B, S, D, M = 4, 8192, 1024, 512
HALF = S // 2          # rows per core
TS = 128               # s-tile rows
NT = HALF // TS        # compute tiles per core (32)
SLAB = HALF + TS       # slab rows incl. halo tile
LN_EPS = 1e-5
N_CORES = 8
ND = D // TS           # 8 feature blocks of x
NM = M // TS           # 4 feature blocks of memory dim
# packed weights: wd(8*512) wkvq(4*1536) w1(4*512) w2(4*512) wu(4*1024) tt(2*128)
WPACK_COLS = ND * M + NM * 3 * M + NM * M + NM * M + NM * D + 2 * TS

f32 = mybir.dt.float32
bf16 = mybir.dt.bfloat16
i32 = mybir.dt.int32
AF = mybir.ActivationFunctionType
ALU = mybir.AluOpType
BF = ml_dtypes.bfloat16

RSQRT_MAGIC = 0x5F3759DF

# test.py can flip these
TRACE = False
TRACE_KWARGS = {}
LAST_RESULTS = None    # BassKernelResults of the last run (exec_time_ns etc.)

_PROG_CACHE = {}


def _fix_matmult_waits(nc):
    """Walrus allows only one sync-wait on a (fused-ldweights) Matmult.
    Move surplus waits onto an inserted NoOp on the same engine."""
    n = 0
    for f in nc.m.functions:
        for bb in f.blocks:
            insts = bb.instructions
            i = 0
            while i < len(insts):
                inst = insts[i]
                si = inst.sync_info
                tname = type(inst).__name__
                exempt = tname in ("InstNoOp",
                                   "InstEventSemaphore",
                                   "InstUnconditionalBranch", "InstCall",
                                   "InstISA", "InstRegisterMove")
                if (not exempt and si is not None and si.on_wait
                        and len(si.on_wait) > 1):
                    for w in list(si.on_wait[:-1]):
                        nop = mybir.InstNoOp(
                            name=f"wfix-{n}", ins=[], outs=[],
                            engine=inst.engine,
                            sync_info=mybir.SyncInfo(on_wait=[w],
                                                     on_update=[]))
                        insts.insert(i, nop)
                        n += 1
                        i += 1
                    si.on_wait = [si.on_wait[-1]]
                i += 1
    return n


def _build_program(flags):
    (has_bk, has_bv, has_bq, has_gq, has_bqln, has_gk, has_bkln,
     has_bu) = flags
    any_kv_bias = has_bk or has_bv
    nc = bass.Bass()

    x_slab = nc.declare_dram_parameter("x_slab", [SLAB, D], bf16,
                                       isOutput=False)
    wpack = nc.declare_dram_parameter("wpack", [TS, WPACK_COLS], bf16,
                                      isOutput=False)
    ident = nc.declare_dram_parameter("ident", [TS, TS], bf16, isOutput=False)
    hmask = nc.declare_dram_parameter("hmask", [TS, 1], f32, isOutput=False)
    opt = {}
    for name, used, shape in (
        ("bk_r", has_bk, [1, M]), ("bv_r", has_bv, [1, M]),
        ("bq_r", has_bq, [1, M]), ("bu_r", has_bu, [1, D]),
        ("gq_b", has_gq, [TS, M]), ("bqln_b", has_bqln, [TS, M]),
        ("gk_b", has_gk, [TS, M]), ("bkln_b", has_bkln, [TS, M]),
    ):
        if used:
            dt = bf16 if name.endswith("_r") else f32
            opt[name] = nc.declare_dram_parameter(name, shape, dt,
                                                  isOutput=False)
    y = nc.declare_dram_parameter("y", [HALF, D], bf16, isOutput=True)

    with tile.TileContext(nc) as tc, ExitStack() as ctx:
        wpool = ctx.enter_context(tc.tile_pool(name="weights", bufs=1))

        wp_sb = wpool.tile([TS, WPACK_COLS], bf16)
        nc.sync.dma_start(wp_sb[:], wpack[:])
        _off = [0]

        def _wseg(nk, ncols):
            a = _off[0]
            _off[0] += nk * ncols
            return wp_sb[:, a:_off[0]].rearrange("p (k m) -> p k m", k=nk)

        wd_sb = _wseg(ND, M)        # [128, dblk, m]    Wd[dblk*128+p, m]
        wkvq_sb = _wseg(NM, 3 * M)  # [128, fblk, kvq]  (k | v | q) cols
        w1_sb = _wseg(NM, M)        # [128, fblk, m1]
        w2_sb = _wseg(NM, M)        # [128, m1blk, m2]
        wu_sb = _wseg(NM, D)        # [128, mblk, d]
        tt_sb = _wseg(2, TS)        # [128, j, t]  Toeplitz (lr folded)
        id_sb = wpool.tile([TS, TS], bf16)
        nc.sync.dma_start(id_sb[:], ident[:])
        hm_sb = wpool.tile([TS, 1], f32)
        nc.sync.dma_start(hm_sb[:], hmask[:])
        magic_sb = wpool.tile([TS, 1], i32)
        nc.vector.memset(magic_sb[:], RSQRT_MAGIC)
        ones_sb = None
        if any_kv_bias or has_bq or has_bu:
            ones_sb = wpool.tile([1, TS], bf16)
            nc.vector.memset(ones_sb[:], 1.0)
        opt_sb = {}
        for name, h in opt.items():
            dt = bf16 if name.endswith("_r") else f32
            t = wpool.tile(list(h.shape), dt, tag=name, name=name)
            nc.sync.dma_start(t[:], h[:])
            opt_sb[name] = t

        # ---- SBUF activation pools (bf16 unless noted) ----
        p_xr = ctx.enter_context(tc.tile_pool(name="xrow", bufs=6))
        p_xT = ctx.enter_context(tc.tile_pool(name="xT", bufs=3))
        p_hT = ctx.enter_context(tc.tile_pool(name="hT", bufs=4))
        p_k = ctx.enter_context(tc.tile_pool(name="k", bufs=2))
        p_kT = ctx.enter_context(tc.tile_pool(name="kT", bufs=3))
        p_a1T = ctx.enter_context(tc.tile_pool(name="a1T", bufs=2))
        p_u = ctx.enter_context(tc.tile_pool(name="u", bufs=4))
        p_q = ctx.enter_context(tc.tile_pool(name="q", bufs=2))
        p_r = ctx.enter_context(tc.tile_pool(name="r", bufs=2))
        p_rT = ctx.enter_context(tc.tile_pool(name="rT", bufs=3))
        p_y = ctx.enter_context(tc.tile_pool(name="y", bufs=3))
        p_sm = ctx.enter_context(tc.tile_pool(name="sm", bufs=3))
        # ---- PSUM pools: C = 5 x 1-bank [128,512] f32, B = 1 x 2-bank ----
        p_C = ctx.enter_context(tc.tile_pool(name="psC", bufs=5,
                                             space="PSUM"))
        p_B = ctx.enter_context(tc.tile_pool(name="psB", bufs=1,
                                             space="PSUM"))

        # ---- per-tile state handed across pipeline stages ----
        st = {}

        def rsqrt_dve(var_ap, tag):
            """rs = 1/sqrt(var + eps) entirely on DVE (bit trick + 2 Newton).
            var_ap: [128,1] f32."""
            veps = p_sm.tile([TS, 1], f32, tag=f"veps{tag}")
            nc.vector.tensor_scalar_add(veps[:], var_ap, LN_EPS)
            vh = p_sm.tile([TS, 1], f32, tag=f"vh{tag}")
            nc.vector.tensor_scalar_mul(vh[:], veps[:], -0.5)
            yi = p_sm.tile([TS, 1], i32, tag=f"yi{tag}")
            # yi = bits >> 1
            nc.vector.tensor_scalar(yi[:], veps[:].bitcast(i32), 1, None,
                                    ALU.logical_shift_right)
            # yi = MAGIC - (bits >> 1)
            nc.vector.tensor_sub(yi[:], magic_sb[:], yi[:])
            yf = yi[:].bitcast(f32)
            t1 = p_sm.tile([TS, 1], f32, tag=f"t1{tag}")
            for _ in range(2):
                nc.vector.tensor_mul(t1[:], yf, yf)           # y*y
                nc.vector.tensor_mul(t1[:], t1[:], vh[:])     # -0.5*v*y*y
                # y = (t1 + 1.5) * y
                nc.vector.scalar_tensor_tensor(yf, t1[:], 1.5, yf,
                                               ALU.add, ALU.mult)
            return yi

        def layernorm(z_ps, gb, bb, tag, out_pool):
            """LN over free dim of z_ps [128, M] (PSUM) -> SBUF bf16 tile."""
            stt = p_sm.tile([TS, 6], f32, tag=f"bnst{tag}")
            nc.vector.bn_stats(stt[:], z_ps[:])
            ag = p_sm.tile([TS, 2], f32, tag=f"bnag{tag}")
            nc.vector.bn_aggr(ag[:], stt[:])
            rs_i = rsqrt_dve(ag[:, 1:2], tag)
            rs = rs_i[:].bitcast(f32)
            nmr = p_sm.tile([TS, 1], f32, tag=f"nmr{tag}")
            nc.vector.scalar_tensor_tensor(nmr[:], ag[:, 0:1], -1.0, rs,
                                           ALU.mult, ALU.mult)
            o = out_pool.tile([TS, M], bf16, tag=tag, name=tag)
            if gb is None and bb is None:
                nc.scalar.activation(o[:], z_ps[:], AF.Identity,
                                     bias=nmr[:], scale=rs)
            else:
                of = out_pool.tile([TS, M], f32, tag=tag + "f")
                nc.scalar.activation(of[:], z_ps[:], AF.Identity,
                                     bias=nmr[:], scale=rs)
                if gb is not None:
                    nc.vector.tensor_mul(of[:], of[:], gb[:])
                if bb is not None:
                    nc.vector.tensor_add(of[:], of[:], bb[:])
                nc.vector.tensor_copy(o[:], of[:])
            return o

        def dma_load(t):
            """Prefetch x row tile + xT blocks for tile t."""
            xT = p_xT.tile([TS, ND, TS], bf16, tag="xT")
            s0 = t * TS
            for d in range(ND):
                nc.sync.dma_start_transpose(
                    xT[:, d, :], x_slab[s0:s0 + TS, d * TS:(d + 1) * TS])
            xr = None
            if t > 0:
                xr = p_xr.tile([TS, D], bf16, tag="xr")
                nc.sync.dma_start(xr[:], x_slab[s0:s0 + TS, :])
            st[t] = {"xT": xT, "xr": xr}

        def s0(t):
            """hT GEMM; k,v GEMMs; LN-k; kT crossbar."""
            d = st[t]
            # hT = Wd.T @ xT : 4 m-blocks x 8 d-steps, N=128
            hT_ps = p_C.tile([TS, M], f32, tag="C", name="hT_ps")
            for mb in range(NM):
                for db in range(ND):
                    nc.tensor.matmul(
                        hT_ps[:, mb * TS:(mb + 1) * TS],
                        wd_sb[:, db, mb * TS:(mb + 1) * TS],
                        d["xT"][:, db, :],
                        start=(db == 0), stop=(db == ND - 1))
            hT = p_hT.tile([TS, NM, TS], bf16, tag="hT")
            for mb in range(NM):
                nc.scalar.copy(hT[:, mb, :], hT_ps[:, mb * TS:(mb + 1) * TS])

            # k, v row-space GEMMs from stationary hT blocks
            k_ps = p_C.tile([TS, M], f32, tag="C", name="k_ps")
            v_ps = p_C.tile([TS, M], f32, tag="C", name="v_ps")
            # v's accumulation group stays open: s1 accumulates -pred into
            # it (W2 negated on host), so u = v - pred lands in PSUM free.
            for fb in range(NM):
                nc.tensor.matmul(k_ps[:], hT[:, fb, :],
                                 wkvq_sb[:, fb, 0:M],
                                 start=(fb == 0),
                                 stop=(fb == NM - 1 and not has_bk))
                nc.tensor.matmul(v_ps[:], hT[:, fb, :],
                                 wkvq_sb[:, fb, M:2 * M],
                                 start=(fb == 0), stop=False)
            if has_bk:
                nc.tensor.matmul(k_ps[:], ones_sb[:, 0:1],
                                 opt_sb["bk_r"][:], start=False, stop=True)
            if has_bv:
                nc.tensor.matmul(v_ps[:], ones_sb[:, 0:1],
                                 opt_sb["bv_r"][:], start=False, stop=False)

            k_sb = layernorm(k_ps, opt_sb.get("gk_b"), opt_sb.get("bkln_b"),
                             "k", p_k)
            kT = p_kT.tile([TS, NM, TS], bf16, tag="kT")
            for fb in range(NM):
                nc.sync.dma_start_transpose(
                    kT[:, fb, :], k_sb[:, fb * TS:(fb + 1) * TS])
            d.update(kT=kT, v_ps=v_ps, hT=hT)

        def s1(t):
            """a1T GEMM + gelu; pred GEMM; u = v - pred."""
            d = st[t]
            halo = (t == 0)
            a1_ps = p_C.tile([TS, M], f32, tag="C", name="a1_ps")
            for mb in range(NM):
                for fb in range(NM):
                    nc.tensor.matmul(
                        a1_ps[:, mb * TS:(mb + 1) * TS],
                        w1_sb[:, fb, mb * TS:(mb + 1) * TS],
                        d["kT"][:, fb, :],
                        start=(fb == 0), stop=(fb == NM - 1))
            a1T = p_a1T.tile([TS, NM, TS], bf16, tag="a1T")
            for mb in range(NM):
                nc.scalar.activation(a1T[:, mb, :],
                                     a1_ps[:, mb * TS:(mb + 1) * TS],
                                     AF.Gelu_apprx_tanh)
            # -pred accumulates straight into v_ps (W2 is negated on host)
            for mb in range(NM):
                nc.tensor.matmul(d["v_ps"][:], a1T[:, mb, :], w2_sb[:, mb, :],
                                 start=False, stop=(mb == NM - 1))
            u_sb = p_u.tile([TS, M], bf16, tag="u")
            if halo:
                nc.vector.tensor_scalar_mul(u_sb[:], d["v_ps"][:],
                                            hm_sb[:, 0:1])
            else:
                nc.vector.tensor_copy(u_sb[:], d["v_ps"][:])
            d["u"] = u_sb

        def s2(t):
            """q GEMM + LN-q; mem GEMM; r = q * mem; rT crossbar."""
            d = st[t]
            q_ps = p_C.tile([TS, M], f32, tag="C", name="q_ps")
            for fb in range(NM):
                nc.tensor.matmul(q_ps[:], d["hT"][:, fb, :],
                                 wkvq_sb[:, fb, 2 * M:3 * M],
                                 start=(fb == 0),
                                 stop=(fb == NM - 1 and not has_bq))
            if has_bq:
                nc.tensor.matmul(q_ps[:], ones_sb[:, 0:1],
                                 opt_sb["bq_r"][:], start=False, stop=True)
            mem_ps = p_C.tile([TS, M], f32, tag="C", name="mem_ps")
            nc.tensor.matmul(mem_ps[:], tt_sb[:, 0, :], st[t - 1]["u"][:],
                             start=True, stop=False)
            nc.tensor.matmul(mem_ps[:], tt_sb[:, 1, :], d["u"][:],
                             start=False, stop=True)
            q_sb = layernorm(q_ps, opt_sb.get("gq_b"), opt_sb.get("bqln_b"),
                             "q", p_q)
            r_sb = p_r.tile([TS, M], bf16, tag="r")
            nc.vector.tensor_mul(r_sb[:], q_sb[:], mem_ps[:])
            rT = p_rT.tile([TS, NM, TS], bf16, tag="rT")
            for mb in range(NM):
                nc.sync.dma_start_transpose(
                    rT[:, mb, :], r_sb[:, mb * TS:(mb + 1) * TS])
            d["rT"] = rT

        def s3(t):
            """out GEMM (+ x residual in-PSUM); y copy; y store."""
            d = st[t]
            out_ps = p_B.tile([TS, D], f32, tag="B", name="out_ps")
            for cb in range(2):
                cols = slice(cb * M, (cb + 1) * M)
                for mb in range(NM):
                    nc.tensor.matmul(out_ps[:, cols], d["rT"][:, mb, :],
                                     wu_sb[:, mb, cols],
                                     start=(mb == 0), stop=False)
                # residual: + I.T @ x_row
                nc.tensor.matmul(out_ps[:, cols], id_sb[:],
                                 d["xr"][:, cols],
                                 start=False, stop=(not has_bu))
                if has_bu:
                    nc.tensor.matmul(out_ps[:, cols], ones_sb[:, 0:1],
                                     opt_sb["bu_r"][:, cols],
                                     start=False, stop=True)
            y_sb = p_y.tile([TS, D], bf16, tag="y")
            nc.scalar.copy(y_sb[:], out_ps[:])
            s0r = (t - 1) * TS
            nc.sync.dma_start(y[s0r:s0r + TS, :], y_sb[:])
            # drop references so pools can recycle
            del st[t]

        for it in range(NT + 5):
            t0 = it
            t1 = it - 1
            t2 = it - 2
            t3 = it - 3
            t4 = it - 4
            if t0 <= NT:
                dma_load(t0)
            if 0 <= t1 <= NT:
                s0(t1)
            if 0 <= t2 <= NT:
                s1(t2)
            if 1 <= t3 <= NT:
                s2(t3)
            if 1 <= t4 <= NT:
                s3(t4)

    _fix_matmult_waits(nc)
    return nc


def _prep_inputs(x, Wd, bd, Wq, bq, Wk, bk, Wv, bv, gq, bq_ln, gk, bk_ln,
                 W1, W2, Wu, bu, adaptive_lr, forget_factor):
    """Host-side: flags, decay matrix, per-core slabs (bf16)."""
    f = np.float32
    bd, bq, bk, bv, bu = (np.asarray(a, f) for a in (bd, bq, bk, bv, bu))
    gq, bq_ln, gk, bk_ln = (np.asarray(a, f) for a in (gq, bq_ln, gk, bk_ln))
    Wd, Wq, Wk, Wv, W1, W2, Wu = (np.asarray(a, f)
                                  for a in (Wd, Wq, Wk, Wv, W1, W2, Wu))
    # fold bd into the k/v/q biases (h = x@Wd + bd only feeds k,v,q)
    if bd.any():
        bk = bk + bd @ Wk
        bv = bv + bd @ Wv
        bq = bq + bd @ Wq
    flags = (bool(bk.any()), bool(bv.any()), bool(bq.any()),
             bool((gq != 1).any()), bool(bq_ln.any()),
             bool((gk != 1).any()), bool(bk_ln.any()), bool(bu.any()))
    (has_bk, has_bv, has_bq, has_gq, has_bqln, has_gk, has_bkln,
     has_bu) = flags

    g = 1.0 / (1.0 + np.exp(-np.float64(forget_factor)))
    lr = np.float64(adaptive_lr)
    t_idx = np.arange(TS)
    lag_cur = t_idx[:, None] - t_idx[None, :]
    Tcur = np.where(lag_cur >= 0, g ** np.maximum(lag_cur, 0), 0.0) * lr
    lag_prev = t_idx[:, None] + TS - t_idx[None, :]
    Tprev = (g ** lag_prev) * lr
    TT = np.concatenate([Tprev, Tcur], axis=1).T.astype(f)  # [256, 128]

    def seg(w):
        w = np.asarray(w, f)          # [K, N] -> [128, nk*N]
        nk = w.shape[0] // TS
        return w.reshape(nk, TS, w.shape[1]).transpose(1, 0, 2).reshape(TS, -1)

    wkvq = np.concatenate([Wk, Wv, Wq], axis=1)   # [512, 1536]
    wpack = np.ascontiguousarray(np.concatenate(
        [seg(w) for w in (Wd, wkvq, W1, -W2, Wu, TT)], axis=1)).astype(BF)
    common = {
        "wpack": wpack,
        "ident": np.eye(TS, dtype=f).astype(BF),
    }
    if has_bk:
        common["bk_r"] = np.ascontiguousarray(bk[None, :]).astype(BF)
    if has_bv:
        common["bv_r"] = np.ascontiguousarray(bv[None, :]).astype(BF)
    if has_bq:
        common["bq_r"] = np.ascontiguousarray(bq[None, :]).astype(BF)
    if has_bu:
        common["bu_r"] = np.ascontiguousarray(bu[None, :]).astype(BF)
    for name, used, vec in (("gq_b", has_gq, gq), ("bqln_b", has_bqln, bq_ln),
                            ("gk_b", has_gk, gk), ("bkln_b", has_bkln, bk_ln)):
        if used:
            common[name] = np.ascontiguousarray(
                np.broadcast_to(vec, (TS, vec.shape[0])), f)

    x = np.asarray(x, f)
    in_maps = []
    for c in range(N_CORES):
        b, sh = c // 2, c % 2
        if sh == 0:
            haloblk = np.zeros((TS, D), f)
            hm = np.zeros((TS, 1), f)
        else:
            haloblk = x[b, HALF - TS:HALF]
            hm = np.ones((TS, 1), f)
        slab = np.concatenate([haloblk, x[b, sh * HALF:(sh + 1) * HALF]],
                              axis=0)
        m = dict(common)
        m["x_slab"] = np.ascontiguousarray(slab).astype(BF)
        m["hmask"] = hm
        in_maps.append(m)
    return flags, in_maps


def kernel(**inputs):
    global LAST_RESULTS
    flags, in_maps = _prep_inputs(**inputs)
    if flags not in _PROG_CACHE:
        _PROG_CACHE[flags] = _build_program(flags)
    nc = _PROG_CACHE[flags]

    res = run_bass_kernel_spmd(nc, in_maps, list(range(N_CORES)),
                               trace=TRACE, trace_kwargs=TRACE_KWARGS)
    LAST_RESULTS = res

    out = np.empty((B, S, D), np.float32)
    for c in range(N_CORES):
        b, sh = c // 2, c % 2
        out[b, sh * HALF:(sh + 1) * HALF] = np.asarray(
            res.results[c]["y"], dtype=np.float32)
    return out


if __name__ == "__main__":
    print("kernel module for AdvancedNeuralMemory; use test.py to run")


# revision 14
# speedup vs baseline: 2.7446x; 2.6754x over previous
"""Trainium2 Bass kernel for AdvancedNeuralMemory (B=4, S=8192, D=1024, M=512).

Math notes
----------
The recurrence  s_t = g * s_{t-1} + u_t  has a *scalar constant* gate
g = sigmoid(forget_factor) ~= 0.525, so  mem_t = sum_{j<=t} g^(t-j) u_j.
g^129 ~ 7e-37, far below fp32 resolution, so a 256-step window is exact:
for 128-row time tiles,
    mem_tile_i = Tprev.T @ u_{i-1} + Tcur.T @ u_i
with host-precomputed decay-Toeplitz matrices (adaptive_lr folded in).
This removes the sequential dependency entirely -> pure matmuls.

Sharding: 8 cores = (batch 0..3) x (seq half 0..1). Each core processes a
[4096, 1024] slab plus a 128-row halo tile (for u_{i-1} of its first tile).
No cross-device communication.

v2 performance design (vs the fp32r baseline):
- all-bf16 datapath (x staged to HBM as bf16; weights bf16; y stored bf16,
  upcast on host).  bf16 matmuls run 1 col/cycle with FWL weight loads.
- no PE transposes: xT comes straight from HBM through the DMA crossbar
  transpose (dma_start_transpose), kT/rT via SBUF->SBUF crossbar DMA.
- transposed-space GEMMs avoid intermediate transposes entirely:
  hT = Wd.T @ xT (32 mm), a1T = W1.T @ kT (16 mm); row-space GEMMs
  (k/v/q from stationary hT, pred from stationary a1T, out from rT).
- the x residual is folded into the out-GEMM PSUM accumulation via an
  identity-stationary matmul (y = x + r @ Wu in one PSUM group).
- LN rsqrt on DVE (bit-trick + 2 Newton steps): the Scalar engine then only
  ever uses Identity/Copy/Gelu_apprx_tanh, which share one activation
  table -> no ACT_TABLE_LOAD swaps.
- 4-stage software pipeline: per loop iteration emits dma(t), s0(t-1),
  s1(t-2), s2(t-3), s3(t-4), so every cross-engine roundtrip has a full
  iteration of PE work in front of it.
"""

import sys
import os

for _p in ("/opt/trn_rl_repo",):
    if _p not in sys.path and os.path.isdir(_p):
        sys.path.insert(0, _p)

from contextlib import ExitStack

import numpy as np
import ml_dtypes

import concourse.bass as bass
import concourse.mybir as mybir
import concourse.tile as tile
from concourse.bass_utils import run_bass_kernel_spmd

B, S, D, M = 4, 8192, 1024, 512
HALF = S // 2          # rows per core
TS = 128               # s-tile rows
NT = HALF // TS        # compute tiles per core (32)
SLAB = HALF + TS       # slab rows incl. halo tile
LN_EPS = 1e-5
N_CORES = 8
ND = D // TS           # 8 feature blocks of x
NM = M // TS           # 4 feature blocks of memory dim
# packed weights: wd(8*512) wkvq(4*1536) w1(4*512) w2(4*512) wu(4*1024) tt(2*128)
WPACK_COLS = ND * M + NM * 3 * M + NM * M + NM * M + NM * D + 2 * TS

f32 = mybir.dt.float32
bf16 = mybir.dt.bfloat16
i32 = mybir.dt.int32
AF = mybir.ActivationFunctionType
ALU = mybir.AluOpType
BF = ml_dtypes.bfloat16

RSQRT_MAGIC = 0x5F3759DF

# test.py can flip these
TRACE = False
TRACE_KWARGS = {}
LAST_RESULTS = None    # BassKernelResults of the last run (exec_time_ns etc.)

_PROG_CACHE = {}


def _fix_matmult_waits(nc):
    """Walrus allows only one sync-wait on a (fused-ldweights) Matmult.
    Move surplus waits onto an inserted NoOp on the same engine."""
    n = 0
    for f in nc.m.functions:
        for bb in f.blocks:
            insts = bb.instructions
            i = 0
            while i < len(insts):
                inst = insts[i]
                si = inst.sync_info
                tname = type(inst).__name__
                exempt = tname in ("InstNoOp",
                                   "InstEventSemaphore",
                                   "InstUnconditionalBranch", "InstCall",
                                   "InstISA", "InstRegisterMove")
                if (not exempt and si is not None and si.on_wait
                        and len(si.on_wait) > 1):
                    for w in list(si.on_wait[:-1]):
                        nop = mybir.InstNoOp(
                            name=f"wfix-{n}", ins=[], outs=[],
                            engine=inst.engine,
                            sync_info=mybir.SyncInfo(on_wait=[w],
                                                     on_update=[]))
                        insts.insert(i, nop)
                        n += 1
                        i += 1
                    si.on_wait = [si.on_wait[-1]]
                i += 1
    return n


def _build_program(flags):
    (has_bk, has_bv, has_bq, has_gq, has_bqln, has_gk, has_bkln,
     has_bu) = flags
    any_kv_bias = has_bk or has_bv
    nc = bass.Bass()

    x_slab = nc.declare_dram_parameter("x_slab", [SLAB, D], bf16,
                                       isOutput=False)
    wpack = nc.declare_dram_parameter("wpack", [TS, WPACK_COLS], bf16,
                                      isOutput=False)
    ident = nc.declare_dram_parameter("ident", [TS, TS], bf16, isOutput=False)
    hmask = nc.declare_dram_parameter("hmask", [TS, 1], f32, isOutput=False)
    opt = {}
    for name, used, shape in (
        ("bk_r", has_bk, [1, M]), ("bv_r", has_bv, [1, M]),
        ("bq_r", has_bq, [1, M]), ("bu_r", has_bu, [1, D]),
        ("gq_b", has_gq, [TS, M]), ("bqln_b", has_bqln, [TS, M]),
        ("gk_b", has_gk, [TS, M]), ("bkln_b", has_bkln, [TS, M]),
    ):
        if used:
            dt = bf16 if name.endswith("_r") else f32
            opt[name] = nc.declare_dram_parameter(name, shape, dt,
                                                  isOutput=False)
    y = nc.declare_dram_parameter("y", [HALF, D], bf16, isOutput=True)

    with tile.TileContext(nc) as tc, ExitStack() as ctx:
        wpool = ctx.enter_context(tc.tile_pool(name="weights", bufs=1))

        wp_sb = wpool.tile([TS, WPACK_COLS], bf16)
        nc.sync.dma_start(wp_sb[:], wpack[:])
        _off = [0]

        def _wseg(nk, ncols):
            a = _off[0]
            _off[0] += nk * ncols
            return wp_sb[:, a:_off[0]].rearrange("p (k m) -> p k m", k=nk)

        wd_sb = _wseg(ND, M)        # [128, dblk, m]    Wd[dblk*128+p, m]
        wkvq_sb = _wseg(NM, 3 * M)  # [128, fblk, kvq]  (k | v | q) cols
        w1_sb = _wseg(NM, M)        # [128, fblk, m1]
        w2_sb = _wseg(NM, M)        # [128, m1blk, m2]
        wu_sb = _wseg(NM, D)        # [128, mblk, d]
        tt_sb = _wseg(2, TS)        # [128, j, t]  Toeplitz (lr folded)
        id_sb = wpool.tile([TS, TS], bf16)
        nc.sync.dma_start(id_sb[:], ident[:])
        hm_sb = wpool.tile([TS, 1], f32)
        nc.sync.dma_start(hm_sb[:], hmask[:])
        magic_sb = wpool.tile([TS, 1], i32)
        nc.vector.memset(magic_sb[:], RSQRT_MAGIC)
        ones_sb = None
        if any_kv_bias or has_bq or has_bu:
            ones_sb = wpool.tile([1, TS], bf16)
            nc.vector.memset(ones_sb[:], 1.0)
        opt_sb = {}
        for name, h in opt.items():
            dt = bf16 if name.endswith("_r") else f32
            t = wpool.tile(list(h.shape), dt, tag=name, name=name)
            nc.sync.dma_start(t[:], h[:])
            opt_sb[name] = t

        # ---- SBUF activation pools (bf16 unless noted) ----
        p_xr = ctx.enter_context(tc.tile_pool(name="xrow", bufs=10))
        p_xT = ctx.enter_context(tc.tile_pool(name="xT", bufs=3))
        p_hT = ctx.enter_context(tc.tile_pool(name="hT", bufs=6))
        p_k = ctx.enter_context(tc.tile_pool(name="k", bufs=3))
        p_kT = ctx.enter_context(tc.tile_pool(name="kT", bufs=3))
        p_a1T = ctx.enter_context(tc.tile_pool(name="a1T", bufs=3))
        p_u = ctx.enter_context(tc.tile_pool(name="u", bufs=4))
        p_q = ctx.enter_context(tc.tile_pool(name="q", bufs=6))
        p_r = ctx.enter_context(tc.tile_pool(name="r", bufs=3))
        p_rT = ctx.enter_context(tc.tile_pool(name="rT", bufs=3))
        p_y = ctx.enter_context(tc.tile_pool(name="y", bufs=3))
        p_sm = ctx.enter_context(tc.tile_pool(name="sm", bufs=3))
        # ---- PSUM pools (8 banks): C = 4 x 1-bank [128,512] f32,
        #      tp = 2 x 1-bank transpose staging, B = 1 x 2-bank out ----
        p_C = ctx.enter_context(tc.tile_pool(name="psC", bufs=4,
                                             space="PSUM"))
        p_tp = ctx.enter_context(tc.tile_pool(name="pstp", bufs=2,
                                              space="PSUM"))
        p_B = ctx.enter_context(tc.tile_pool(name="psB", bufs=1,
                                             space="PSUM"))

        # ---- per-tile state handed across pipeline stages ----
        st = {}

        def rsqrt_dve(var_ap, tag):
            """rs = 1/sqrt(var + eps) entirely on DVE (bit trick + 2 Newton).
            var_ap: [128,1] f32."""
            veps = p_sm.tile([TS, 1], f32, tag=f"veps{tag}")
            nc.vector.tensor_scalar_add(veps[:], var_ap, LN_EPS)
            vh = p_sm.tile([TS, 1], f32, tag=f"vh{tag}")
            nc.vector.tensor_scalar_mul(vh[:], veps[:], -0.5)
            yi = p_sm.tile([TS, 1], i32, tag=f"yi{tag}")
            # yi = bits >> 1
            nc.vector.tensor_scalar(yi[:], veps[:].bitcast(i32), 1, None,
                                    ALU.logical_shift_right)
            # yi = MAGIC - (bits >> 1)
            nc.vector.tensor_sub(yi[:], magic_sb[:], yi[:])
            yf = yi[:].bitcast(f32)
            t1 = p_sm.tile([TS, 1], f32, tag=f"t1{tag}")
            for _ in range(2):
                nc.vector.tensor_mul(t1[:], yf, yf)           # y*y
                nc.vector.tensor_mul(t1[:], t1[:], vh[:])     # -0.5*v*y*y
                # y = (t1 + 1.5) * y
                nc.vector.scalar_tensor_tensor(yf, t1[:], 1.5, yf,
                                               ALU.add, ALU.mult)
            return yi

        def layernorm(z_ps, gb, bb, tag, out_pool):
            """LN over free dim of z_ps [128, M] (PSUM) -> SBUF bf16 tile."""
            stt = p_sm.tile([TS, 6], f32, tag=f"bnst{tag}")
            nc.vector.bn_stats(stt[:], z_ps[:])
            ag = p_sm.tile([TS, 2], f32, tag=f"bnag{tag}")
            nc.vector.bn_aggr(ag[:], stt[:])
            rs_i = rsqrt_dve(ag[:, 1:2], tag)
            rs = rs_i[:].bitcast(f32)
            nmr = p_sm.tile([TS, 1], f32, tag=f"nmr{tag}")
            nc.vector.scalar_tensor_tensor(nmr[:], ag[:, 0:1], -1.0, rs,
                                           ALU.mult, ALU.mult)
            o = out_pool.tile([TS, M], bf16, tag=tag, name=tag)
            if gb is None and bb is None:
                nc.scalar.activation(o[:], z_ps[:], AF.Identity,
                                     bias=nmr[:], scale=rs)
            else:
                of = out_pool.tile([TS, M], f32, tag=tag + "f")
                nc.scalar.activation(of[:], z_ps[:], AF.Identity,
                                     bias=nmr[:], scale=rs)
                if gb is not None:
                    nc.vector.tensor_mul(of[:], of[:], gb[:])
                if bb is not None:
                    nc.vector.tensor_add(of[:], of[:], bb[:])
                nc.vector.tensor_copy(o[:], of[:])
            return o

        def pe_transpose(src_ap, nblk, out_pool, tag):
            """nblk [128,128] transposes -> one PSUM staging tile (bf16) ->
            one wide copy into an SBUF tile of the same layout."""
            tp = p_tp.tile([TS, 8 * TS], bf16, tag="tp", name=f"tp_{tag}")
            for b in range(nblk):
                nc.tensor.transpose(tp[:, b * TS:(b + 1) * TS],
                                    src_ap[:, b * TS:(b + 1) * TS],
                                    id_sb[:])
            dst = out_pool.tile([TS, nblk, TS], bf16, tag=tag)
            flat = dst[:].rearrange("p a b -> p (a b)")
            if nblk > 4:
                nc.scalar.copy(flat, tp[:, 0:nblk * TS])
            else:
                nc.vector.tensor_copy(flat, tp[:, 0:nblk * TS])
            return dst

        def dma_load(t):
            """Prefetch x row tile for tile t (3 iterations ahead)."""
            xr = p_xr.tile([TS, D], bf16, tag="xr")
            nc.sync.dma_start(xr[:], x_slab[t * TS:(t + 1) * TS, :])
            st[t] = {"xr": xr}

        def tx(t):
            """PE-transpose x(t) -> xT_sb."""
            st[t]["xT"] = pe_transpose(st[t]["xr"][:], ND, p_xT, "xT")

        def s0(t):
            """hT = Wd.T @ xT (transposed-space GEMM) + wide copy."""
            d = st[t]
            hT_ps = p_C.tile([TS, M], f32, tag="C", name="hT_ps")
            for mb in range(NM):
                for db in range(ND):
                    nc.tensor.matmul(
                        hT_ps[:, mb * TS:(mb + 1) * TS],
                        wd_sb[:, db, mb * TS:(mb + 1) * TS],
                        d["xT"][:, db, :],
                        start=(db == 0), stop=(db == ND - 1))
            hT = p_hT.tile([TS, NM, TS], bf16, tag="hT")
            nc.scalar.copy(hT[:].rearrange("p a b -> p (a b)"), hT_ps[:])
            d["hT"] = hT

        def s1(t):
            """k and q GEMMs from stationary hT; LN both."""
            d = st[t]
            k_ps = p_C.tile([TS, M], f32, tag="C", name="k_ps")
            q_ps = p_C.tile([TS, M], f32, tag="C", name="q_ps")
            for fb in range(NM):
                nc.tensor.matmul(k_ps[:], d["hT"][:, fb, :],
                                 wkvq_sb[:, fb, 0:M],
                                 start=(fb == 0),
                                 stop=(fb == NM - 1 and not has_bk))
                nc.tensor.matmul(q_ps[:], d["hT"][:, fb, :],
                                 wkvq_sb[:, fb, 2 * M:3 * M],
                                 start=(fb == 0),
                                 stop=(fb == NM - 1 and not has_bq))
            if has_bk:
                nc.tensor.matmul(k_ps[:], ones_sb[:, 0:1],
                                 opt_sb["bk_r"][:], start=False, stop=True)
            if has_bq:
                nc.tensor.matmul(q_ps[:], ones_sb[:, 0:1],
                                 opt_sb["bq_r"][:], start=False, stop=True)
            d["k"] = layernorm(k_ps, opt_sb.get("gk_b"),
                               opt_sb.get("bkln_b"), "k", p_k)
            d["q"] = layernorm(q_ps, opt_sb.get("gq_b"),
                               opt_sb.get("bqln_b"), "q", p_q)

        def s2(t):
            """kT PE transpose."""
            st[t]["kT"] = pe_transpose(st[t]["k"][:], NM, p_kT, "kT")

        def s3(t):
            """a1T = gelu(W1.T @ kT) transposed-space GEMM."""
            d = st[t]
            a1_ps = p_tp.tile([TS, M], f32, tag="tp", name="a1_ps")
            for mb in range(NM):
                for fb in range(NM):
                    nc.tensor.matmul(
                        a1_ps[:, mb * TS:(mb + 1) * TS],
                        w1_sb[:, fb, mb * TS:(mb + 1) * TS],
                        d["kT"][:, fb, :],
                        start=(fb == 0), stop=(fb == NM - 1))
            a1T = p_a1T.tile([TS, NM, TS], bf16, tag="a1T")
            nc.scalar.activation(a1T[:].rearrange("p a b -> p (a b)"),
                                 a1_ps[:], AF.Gelu_apprx_tanh)
            d["a1T"] = a1T

        def s4(t):
            """up = -pred + v in one PSUM group (W2 negated on host);
            u = copy(up) [* halo mask]."""
            d = st[t]
            halo = (t == 0)
            up = p_C.tile([TS, M], f32, tag="C", name="up")
            for mb in range(NM):
                nc.tensor.matmul(up[:], d["a1T"][:, mb, :], w2_sb[:, mb, :],
                                 start=(mb == 0), stop=False)
            for fb in range(NM):
                nc.tensor.matmul(up[:], d["hT"][:, fb, :],
                                 wkvq_sb[:, fb, M:2 * M],
                                 start=False,
                                 stop=(fb == NM - 1 and not has_bv))
            if has_bv:
                nc.tensor.matmul(up[:], ones_sb[:, 0:1],
                                 opt_sb["bv_r"][:], start=False, stop=True)
            u_sb = p_u.tile([TS, M], bf16, tag="u")
            if halo:
                nc.vector.tensor_scalar_mul(u_sb[:], up[:], hm_sb[:, 0:1])
            else:
                nc.vector.tensor_copy(u_sb[:], up[:])
            d["u"] = u_sb

        def s5(t):
            """mem Toeplitz GEMM; r = q * mem."""
            d = st[t]
            mem_ps = p_C.tile([TS, M], f32, tag="C", name="mem_ps")
            nc.tensor.matmul(mem_ps[:], tt_sb[:, 0, :], st[t - 1]["u"][:],
                             start=True, stop=False)
            nc.tensor.matmul(mem_ps[:], tt_sb[:, 1, :], d["u"][:],
                             start=False, stop=True)
            r_sb = p_r.tile([TS, M], bf16, tag="r")
            nc.vector.tensor_mul(r_sb[:], d["q"][:], mem_ps[:])
            d["r"] = r_sb

        def s6(t):
            """rT PE transpose."""
            st[t]["rT"] = pe_transpose(st[t]["r"][:], NM, p_rT, "rT")

        def s7(t):
            """out GEMM (+ x residual in-PSUM); y copy; y store."""
            d = st[t]
            out_ps = p_B.tile([TS, D], f32, tag="B", name="out_ps")
            for cb in range(2):
                cols = slice(cb * M, (cb + 1) * M)
                for mb in range(NM):
                    nc.tensor.matmul(out_ps[:, cols], d["rT"][:, mb, :],
                                     wu_sb[:, mb, cols],
                                     start=(mb == 0), stop=False)
                # residual: + I.T @ x_row
                nc.tensor.matmul(out_ps[:, cols], id_sb[:],
                                 d["xr"][:, cols],
                                 start=False, stop=(not has_bu))
                if has_bu:
                    nc.tensor.matmul(out_ps[:, cols], ones_sb[:, 0:1],
                                     opt_sb["bu_r"][:, cols],
                                     start=False, stop=True)
            y_sb = p_y.tile([TS, D], bf16, tag="y")
            nc.scalar.copy(y_sb[:], out_ps[:])
            s0r = (t - 1) * TS
            nc.sync.dma_start(y[s0r:s0r + TS, :], y_sb[:])
            # drop references so pools can recycle
            del st[t]

        stages = (
            (dma_load, 0, 0), (tx, 1, 0), (s0, 2, 0), (s1, 3, 0),
            (s2, 4, 0), (s3, 5, 0), (s4, 6, 0), (s5, 7, 1), (s6, 8, 1),
            (s7, 9, 1),
        )
        for it in range(NT + 10):
            for fn, lag, tmin in stages:
                t = it - lag
                if tmin <= t <= NT:
                    fn(t)

    _fix_matmult_waits(nc)
    return nc


def _prep_inputs(x, Wd, bd, Wq, bq, Wk, bk, Wv, bv, gq, bq_ln, gk, bk_ln,
                 W1, W2, Wu, bu, adaptive_lr, forget_factor):
    """Host-side: flags, decay matrix, per-core slabs (bf16)."""
    f = np.float32
    bd, bq, bk, bv, bu = (np.asarray(a, f) for a in (bd, bq, bk, bv, bu))
    gq, bq_ln, gk, bk_ln = (np.asarray(a, f) for a in (gq, bq_ln, gk, bk_ln))
    Wd, Wq, Wk, Wv, W1, W2, Wu = (np.asarray(a, f)
                                  for a in (Wd, Wq, Wk, Wv, W1, W2, Wu))
    # fold bd into the k/v/q biases (h = x@Wd + bd only feeds k,v,q)
    if bd.any():
        bk = bk + bd @ Wk
        bv = bv + bd @ Wv
        bq = bq + bd @ Wq
    flags = (bool(bk.any()), bool(bv.any()), bool(bq.any()),
             bool((gq != 1).any()), bool(bq_ln.any()),
             bool((gk != 1).any()), bool(bk_ln.any()), bool(bu.any()))
    (has_bk, has_bv, has_bq, has_gq, has_bqln, has_gk, has_bkln,
     has_bu) = flags

    g = 1.0 / (1.0 + np.exp(-np.float64(forget_factor)))
    lr = np.float64(adaptive_lr)
    t_idx = np.arange(TS)
    lag_cur = t_idx[:, None] - t_idx[None, :]
    Tcur = np.where(lag_cur >= 0, g ** np.maximum(lag_cur, 0), 0.0) * lr
    lag_prev = t_idx[:, None] + TS - t_idx[None, :]
    Tprev = (g ** lag_prev) * lr
    TT = np.concatenate([Tprev, Tcur], axis=1).T.astype(f)  # [256, 128]

    def seg(w):
        w = np.asarray(w, f)          # [K, N] -> [128, nk*N]
        nk = w.shape[0] // TS
        return w.reshape(nk, TS, w.shape[1]).transpose(1, 0, 2).reshape(TS, -1)

    wkvq = np.concatenate([Wk, Wv, Wq], axis=1)   # [512, 1536]
    wpack = np.ascontiguousarray(np.concatenate(
        [seg(w) for w in (Wd, wkvq, W1, -W2, Wu, TT)], axis=1)).astype(BF)
    common = {
        "wpack": wpack,
        "ident": np.eye(TS, dtype=f).astype(BF),
    }
    if has_bk:
        common["bk_r"] = np.ascontiguousarray(bk[None, :]).astype(BF)
    if has_bv:
        common["bv_r"] = np.ascontiguousarray(bv[None, :]).astype(BF)
    if has_bq:
        common["bq_r"] = np.ascontiguousarray(bq[None, :]).astype(BF)
    if has_bu:
        common["bu_r"] = np.ascontiguousarray(bu[None, :]).astype(BF)
    for name, used, vec in (("gq_b", has_gq, gq), ("bqln_b", has_bqln, bq_ln),
                            ("gk_b", has_gk, gk), ("bkln_b", has_bkln, bk_ln)):
        if used:
            common[name] = np.ascontiguousarray(
                np.broadcast_to(vec, (TS, vec.shape[0])), f)

    x = np.asarray(x, f)
    in_maps = []
    for c in range(N_CORES):
        b, sh = c // 2, c % 2
        if sh == 0:
            haloblk = np.zeros((TS, D), f)
            hm = np.zeros((TS, 1), f)
        else:
            haloblk = x[b, HALF - TS:HALF]
            hm = np.ones((TS, 1), f)
        slab = np.concatenate([haloblk, x[b, sh * HALF:(sh + 1) * HALF]],
                              axis=0)
        m = dict(common)
        m["x_slab"] = np.ascontiguousarray(slab).astype(BF)
        m["hmask"] = hm
        in_maps.append(m)
    return flags, in_maps


def kernel(**inputs):
    global LAST_RESULTS
    flags, in_maps = _prep_inputs(**inputs)
    if flags not in _PROG_CACHE:
        _PROG_CACHE[flags] = _build_program(flags)
    nc = _PROG_CACHE[flags]

    res = run_bass_kernel_spmd(nc, in_maps, list(range(N_CORES)),
                               trace=TRACE, trace_kwargs=TRACE_KWARGS)
    LAST_RESULTS = res

    out = np.empty((B, S, D), np.float32)
    for c in range(N_CORES):
        b, sh = c // 2, c % 2
        out[b, sh * HALF:(sh + 1) * HALF] = np.asarray(
            res.results[c]["y"], dtype=np.float32)
    return out


if __name__ == "__main__":
    print("kernel module for AdvancedNeuralMemory; use test.py to run")


# revision 24
# speedup vs baseline: 3.5714x; 1.3013x over previous
"""Trainium2 Bass kernel for AdvancedNeuralMemory (B=4, S=8192, D=1024, M=512).

Math notes
----------
The recurrence  s_t = g * s_{t-1} + u_t  has a *scalar constant* gate
g = sigmoid(forget_factor) ~= 0.525, so  mem_t = sum_{j<=t} g^(t-j) u_j.
g^129 ~ 7e-37, far below fp32 resolution, so a 256-step window is exact:
for 128-row time tiles,
    mem_tile_i = Tprev.T @ u_{i-1} + Tcur.T @ u_i
with host-precomputed decay-Toeplitz matrices (adaptive_lr folded in).
This removes the sequential dependency entirely -> pure matmuls.

Sharding: 8 cores = (batch 0..3) x (seq half 0..1). Each core processes a
[4096, 1024] slab plus a 128-row halo tile (for u_{i-1} of its first tile).
No cross-device communication.

v2 performance design (vs the fp32r baseline):
- all-bf16 datapath (x staged to HBM as bf16; weights bf16; y stored bf16,
  upcast on host).  bf16 matmuls run 1 col/cycle with FWL weight loads.
- no PE transposes: xT comes straight from HBM through the DMA crossbar
  transpose (dma_start_transpose), kT/rT via SBUF->SBUF crossbar DMA.
- transposed-space GEMMs avoid intermediate transposes entirely:
  hT = Wd.T @ xT (32 mm), a1T = W1.T @ kT (16 mm); row-space GEMMs
  (k/v/q from stationary hT, pred from stationary a1T, out from rT).
- the x residual is folded into the out-GEMM PSUM accumulation via an
  identity-stationary matmul (y = x + r @ Wu in one PSUM group).
- LN rsqrt on DVE (bit-trick + 2 Newton steps): the Scalar engine then only
  ever uses Identity/Copy/Gelu_apprx_tanh, which share one activation
  table -> no ACT_TABLE_LOAD swaps.
- 4-stage software pipeline: per loop iteration emits dma(t), s0(t-1),
  s1(t-2), s2(t-3), s3(t-4), so every cross-engine roundtrip has a full
  iteration of PE work in front of it.
"""

import sys
import os

for _p in ("/opt/trn_rl_repo",):
    if _p not in sys.path and os.path.isdir(_p):
        sys.path.insert(0, _p)

from contextlib import ExitStack

import numpy as np
import ml_dtypes

import concourse.bass as bass
import concourse.mybir as mybir
import concourse.tile as tile
from concourse.bass_utils import run_bass_kernel_spmd

B, S, D, M = 4, 8192, 1024, 512
HALF = S // 2          # rows per core
TS = 128               # s-tile rows
NT = HALF // TS        # compute tiles per core (32)
SLAB = HALF + TS       # slab rows incl. halo tile
LN_EPS = 1e-5
N_CORES = 8
ND = D // TS           # 8 feature blocks of x
NM = M // TS           # 4 feature blocks of memory dim

f32 = mybir.dt.float32
bf16 = mybir.dt.bfloat16
fp8 = mybir.dt.float8e4
i32 = mybir.dt.int32
AF = mybir.ActivationFunctionType
ALU = mybir.AluOpType
DR = mybir.MatmulPerfMode.DoubleRow
BF = ml_dtypes.bfloat16
F8 = ml_dtypes.float8_e4m3fn

RSQRT_MAGIC = 0x5F3759DF

# test.py can flip these
TRACE = False
TRACE_KWARGS = {}
LAST_RESULTS = None    # BassKernelResults of the last run (exec_time_ns etc.)

_PROG_CACHE = {}


def _fix_matmult_waits(nc):
    """Walrus allows only one sync-wait on a (fused-ldweights) Matmult.
    Move surplus waits onto an inserted NoOp on the same engine."""
    n = 0
    for f in nc.m.functions:
        for bb in f.blocks:
            insts = bb.instructions
            i = 0
            while i < len(insts):
                inst = insts[i]
                si = inst.sync_info
                tname = type(inst).__name__
                exempt = tname in ("InstNoOp",
                                   "InstEventSemaphore",
                                   "InstUnconditionalBranch", "InstCall",
                                   "InstISA", "InstRegisterMove")
                if (not exempt and si is not None and si.on_wait
                        and len(si.on_wait) > 1):
                    for w in list(si.on_wait[:-1]):
                        nop = mybir.InstNoOp(
                            name=f"wfix-{n}", ins=[], outs=[],
                            engine=inst.engine,
                            sync_info=mybir.SyncInfo(on_wait=[w],
                                                     on_update=[]))
                        insts.insert(i, nop)
                        n += 1
                        i += 1
                    si.on_wait = [si.on_wait[-1]]
                i += 1
    return n


def _build_program(flags):
    (has_bk, has_bv, has_bq, has_gq, has_bqln, has_gk, has_bkln,
     has_bu) = flags
    any_kv_bias = has_bk or has_bv
    nc = bass.Bass()

    x_slab = nc.declare_dram_parameter("x_slab", [SLAB, D], bf16,
                                       isOutput=False)
    wseg_specs = (  # name, dtype, shape [TS, nk, cols]
        ("wd", bf16, [TS, ND, M]),
        ("wkvq", fp8, [TS, NM, 3 * M]),
        ("w1", bf16, [TS, NM, M]),
        ("w2", fp8, [TS, NM, M]),
        ("wu", fp8, [TS, NM, D]),
        ("tt", bf16, [TS, 2, TS]),
    )
    wseg = {name: nc.declare_dram_parameter(name, shape, dt, isOutput=False)
            for name, dt, shape in wseg_specs}
    ident = nc.declare_dram_parameter("ident", [TS, TS], bf16, isOutput=False)
    hmask = nc.declare_dram_parameter("hmask", [TS, 1], f32, isOutput=False)
    opt = {}
    for name, used, shape in (
        ("bk_r", has_bk, [1, M]), ("bv_r", has_bv, [1, M]),
        ("bq_r", has_bq, [1, M]), ("bu_r", has_bu, [1, D]),
        ("gq_b", has_gq, [TS, M]), ("bqln_b", has_bqln, [TS, M]),
        ("gk_b", has_gk, [TS, M]), ("bkln_b", has_bkln, [TS, M]),
    ):
        if used:
            dt = bf16 if name.endswith("_r") else f32
            opt[name] = nc.declare_dram_parameter(name, shape, dt,
                                                  isOutput=False)
    y = nc.declare_dram_parameter("y", [HALF, D], bf16, isOutput=True)

    with tile.TileContext(nc) as tc, ExitStack() as ctx:
        wpool = ctx.enter_context(tc.tile_pool(name="weights", bufs=1))

        wsb = {}
        for name, dt, shape in wseg_specs:
            t = wpool.tile(shape, dt, tag=name, name=name)
            nc.sync.dma_start(t[:], wseg[name][:])
            wsb[name] = t
        wd_sb = wsb["wd"]       # [128, dblk, m]    Wd[dblk*128+p, m]
        wkvq_sb = wsb["wkvq"]   # [128, fblk, kvq]  (k | v | q) cols, fp8
        w1_sb = wsb["w1"]       # [128, fblk, m1]
        w2_sb = wsb["w2"]       # [128, m1blk, m2]  (-W2), fp8
        wu_sb = wsb["wu"]       # [128, mblk, d], fp8
        tt_sb = wsb["tt"]       # [128, j, t]  Toeplitz (lr folded)
        id_sb = wpool.tile([TS, TS], bf16)
        nc.sync.dma_start(id_sb[:], ident[:])
        hm_sb = wpool.tile([TS, 1], f32)
        nc.sync.dma_start(hm_sb[:], hmask[:])
        magic_sb = wpool.tile([TS, 1], i32)
        nc.vector.memset(magic_sb[:], RSQRT_MAGIC)
        ones_sb = None
        if any_kv_bias or has_bq or has_bu:
            ones_sb = wpool.tile([1, TS], bf16)
            nc.vector.memset(ones_sb[:], 1.0)
        opt_sb = {}
        for name, h in opt.items():
            dt = bf16 if name.endswith("_r") else f32
            t = wpool.tile(list(h.shape), dt, tag=name, name=name)
            nc.sync.dma_start(t[:], h[:])
            opt_sb[name] = t

        # ---- SBUF activation pools (bf16 unless noted) ----
        p_xr = ctx.enter_context(tc.tile_pool(name="xrow", bufs=10))
        p_xT = ctx.enter_context(tc.tile_pool(name="xT", bufs=3))
        p_hT = ctx.enter_context(tc.tile_pool(name="hT", bufs=6))
        p_k = ctx.enter_context(tc.tile_pool(name="k", bufs=3))
        p_kT = ctx.enter_context(tc.tile_pool(name="kT", bufs=3))
        p_a1T = ctx.enter_context(tc.tile_pool(name="a1T", bufs=3))
        p_u = ctx.enter_context(tc.tile_pool(name="u", bufs=4))
        p_q = ctx.enter_context(tc.tile_pool(name="q", bufs=6))
        p_r = ctx.enter_context(tc.tile_pool(name="r", bufs=3))
        p_rT = ctx.enter_context(tc.tile_pool(name="rT", bufs=3))
        p_y = ctx.enter_context(tc.tile_pool(name="y", bufs=3))
        p_sm = ctx.enter_context(tc.tile_pool(name="sm", bufs=3))
        # ---- PSUM pools (8 banks): C = 4 x 1-bank [128,512] f32,
        #      tp = 2 x 1-bank transpose staging, B = 1 x 2-bank out ----
        p_C = ctx.enter_context(tc.tile_pool(name="psC", bufs=4,
                                             space="PSUM"))
        p_tp = ctx.enter_context(tc.tile_pool(name="pstp", bufs=2,
                                              space="PSUM"))
        p_B = ctx.enter_context(tc.tile_pool(name="psB", bufs=1,
                                             space="PSUM"))

        # ---- per-tile state handed across pipeline stages ----
        st = {}

        def rsqrt_dve(var_ap, tag):
            """rs = 1/sqrt(var + eps) entirely on DVE (bit trick + 2 Newton).
            var_ap: [128,1] f32."""
            veps = p_sm.tile([TS, 1], f32, tag=f"veps{tag}")
            nc.vector.tensor_scalar_add(veps[:], var_ap, LN_EPS)
            vh = p_sm.tile([TS, 1], f32, tag=f"vh{tag}")
            nc.vector.tensor_scalar_mul(vh[:], veps[:], -0.5)
            yi = p_sm.tile([TS, 1], i32, tag=f"yi{tag}")
            # yi = bits >> 1
            nc.vector.tensor_scalar(yi[:], veps[:].bitcast(i32), 1, None,
                                    ALU.logical_shift_right)
            # yi = MAGIC - (bits >> 1)
            nc.vector.tensor_sub(yi[:], magic_sb[:], yi[:])
            yf = yi[:].bitcast(f32)
            t1 = p_sm.tile([TS, 1], f32, tag=f"t1{tag}")
            for _ in range(2):
                nc.vector.tensor_mul(t1[:], yf, yf)           # y*y
                nc.vector.tensor_mul(t1[:], t1[:], vh[:])     # -0.5*v*y*y
                # y = (t1 + 1.5) * y
                nc.vector.scalar_tensor_tensor(yf, t1[:], 1.5, yf,
                                               ALU.add, ALU.mult)
            return yi

        def layernorm(z_ps, gb, bb, tag, out_pool):
            """LN over free dim of z_ps [128, M] (PSUM) -> SBUF bf16 tile."""
            stt = p_sm.tile([TS, 6], f32, tag=f"bnst{tag}")
            nc.vector.bn_stats(stt[:], z_ps[:])
            ag = p_sm.tile([TS, 2], f32, tag=f"bnag{tag}")
            nc.vector.bn_aggr(ag[:], stt[:])
            rs_i = rsqrt_dve(ag[:, 1:2], tag)
            rs = rs_i[:].bitcast(f32)
            nmr = p_sm.tile([TS, 1], f32, tag=f"nmr{tag}")
            nc.vector.scalar_tensor_tensor(nmr[:], ag[:, 0:1], -1.0, rs,
                                           ALU.mult, ALU.mult)
            o = out_pool.tile([TS, M], bf16, tag=tag, name=tag)
            if gb is None and bb is None:
                nc.scalar.activation(o[:], z_ps[:], AF.Identity,
                                     bias=nmr[:], scale=rs)
            else:
                of = out_pool.tile([TS, M], f32, tag=tag + "f")
                nc.scalar.activation(of[:], z_ps[:], AF.Identity,
                                     bias=nmr[:], scale=rs)
                if gb is not None:
                    nc.vector.tensor_mul(of[:], of[:], gb[:])
                if bb is not None:
                    nc.vector.tensor_add(of[:], of[:], bb[:])
                nc.vector.tensor_copy(o[:], of[:])
            return o

        def pe_transpose(src_ap, nblk, out_pool, tag, dst_dt=bf16):
            """nblk [128,128] transposes -> one PSUM staging tile (bf16) ->
            one wide copy (with cast) into an SBUF tile."""
            tp = p_tp.tile([TS, 8 * TS], bf16, tag="tp", name=f"tp_{tag}")
            for b in range(nblk):
                nc.tensor.transpose(tp[:, b * TS:(b + 1) * TS],
                                    src_ap[:, b * TS:(b + 1) * TS],
                                    id_sb[:])
            dst = out_pool.tile([TS, nblk, TS], dst_dt, tag=tag)
            flat = dst[:].rearrange("p a b -> p (a b)")
            if nblk > 4:
                nc.scalar.copy(flat, tp[:, 0:nblk * TS])
            else:
                nc.vector.tensor_copy(flat, tp[:, 0:nblk * TS])
            return dst

        def dma_load(t):
            """Prefetch x row tile for tile t (3 iterations ahead)."""
            xr = p_xr.tile([TS, D], bf16, tag="xr")
            nc.sync.dma_start(xr[:], x_slab[t * TS:(t + 1) * TS, :])
            st[t] = {"xr": xr}

        def tx(t):
            """PE-transpose x(t) -> xT_sb."""
            st[t]["xT"] = pe_transpose(st[t]["xr"][:], ND, p_xT, "xT")

        def s0(t):
            """hT = Wd.T @ xT (transposed-space GEMM) + wide copy."""
            d = st[t]
            hT_ps = p_C.tile([TS, M], f32, tag="C", name="hT_ps")
            for mb in range(NM):
                for db in range(ND):
                    nc.tensor.matmul(
                        hT_ps[:, mb * TS:(mb + 1) * TS],
                        wd_sb[:, db, mb * TS:(mb + 1) * TS],
                        d["xT"][:, db, :],
                        start=(db == 0), stop=(db == ND - 1))
            hT = p_hT.tile([TS, NM, TS], fp8, tag="hT")
            nc.scalar.copy(hT[:].rearrange("p a b -> p (a b)"), hT_ps[:])
            d["hT"] = hT

        def s1(t):
            """k and q GEMMs (fp8 DoubleRow) from stationary hT; LN both.
            Weights are host-scaled x16 for fp8 range; LN absorbs it."""
            d = st[t]
            k_ps = p_C.tile([TS, M], f32, tag="C", name="k_ps")
            q_ps = p_C.tile([TS, M], f32, tag="C", name="q_ps")
            for c in range(2):
                pr = slice(2 * c, 2 * c + 2)
                nc.tensor.matmul(k_ps[:], d["hT"][:, pr, :],
                                 wkvq_sb[:, pr, 0:M], perf_mode=DR,
                                 start=(c == 0),
                                 stop=(c == 1 and not has_bk))
                nc.tensor.matmul(q_ps[:], d["hT"][:, pr, :],
                                 wkvq_sb[:, pr, 2 * M:3 * M], perf_mode=DR,
                                 start=(c == 0),
                                 stop=(c == 1 and not has_bq))
            if has_bk:
                nc.tensor.matmul(k_ps[:], ones_sb[:, 0:1],
                                 opt_sb["bk_r"][:], start=False, stop=True)
            if has_bq:
                nc.tensor.matmul(q_ps[:], ones_sb[:, 0:1],
                                 opt_sb["bq_r"][:], start=False, stop=True)
            d["k"] = layernorm(k_ps, opt_sb.get("gk_b"),
                               opt_sb.get("bkln_b"), "k", p_k)
            d["q"] = layernorm(q_ps, opt_sb.get("gq_b"),
                               opt_sb.get("bqln_b"), "q", p_q)

        def s2(t):
            """kT PE transpose."""
            st[t]["kT"] = pe_transpose(st[t]["k"][:], NM, p_kT, "kT")

        def s3(t):
            """a1T = gelu(W1.T @ kT) transposed-space GEMM."""
            d = st[t]
            a1_ps = p_tp.tile([TS, M], f32, tag="tp", name="a1_ps")
            for mb in range(NM):
                for fb in range(NM):
                    nc.tensor.matmul(
                        a1_ps[:, mb * TS:(mb + 1) * TS],
                        w1_sb[:, fb, mb * TS:(mb + 1) * TS],
                        d["kT"][:, fb, :],
                        start=(fb == 0), stop=(fb == NM - 1))
            a1T = p_a1T.tile([TS, NM, TS], fp8, tag="a1T")
            nc.scalar.activation(a1T[:].rearrange("p a b -> p (a b)"),
                                 a1_ps[:], AF.Gelu_apprx_tanh)
            d["a1T"] = a1T

        def s4(t):
            """up = 16*(v - pred) in one PSUM group, fp8 DoubleRow
            (W2 negated and x16 on host; 1/16 folded into the Toeplitz);
            u = copy(up) [* halo mask]."""
            d = st[t]
            halo = (t == 0)
            up = p_C.tile([TS, M], f32, tag="C", name="up")
            for c in range(2):
                pr = slice(2 * c, 2 * c + 2)
                nc.tensor.matmul(up[:], d["a1T"][:, pr, :],
                                 w2_sb[:, pr, :], perf_mode=DR,
                                 start=(c == 0), stop=False)
            for c in range(2):
                pr = slice(2 * c, 2 * c + 2)
                nc.tensor.matmul(up[:], d["hT"][:, pr, :],
                                 wkvq_sb[:, pr, M:2 * M], perf_mode=DR,
                                 start=False,
                                 stop=(c == 1 and not has_bv))
            if has_bv:
                nc.tensor.matmul(up[:], ones_sb[:, 0:1],
                                 opt_sb["bv_r"][:], start=False, stop=True)
            u_sb = p_u.tile([TS, M], bf16, tag="u")
            if halo:
                nc.vector.tensor_scalar_mul(u_sb[:], up[:], hm_sb[:, 0:1])
            else:
                nc.vector.tensor_copy(u_sb[:], up[:])
            d["u"] = u_sb

        def s5(t):
            """mem Toeplitz GEMM; r = q * mem."""
            d = st[t]
            mem_ps = p_C.tile([TS, M], f32, tag="C", name="mem_ps")
            nc.tensor.matmul(mem_ps[:], tt_sb[:, 0, :], st[t - 1]["u"][:],
                             start=True, stop=False)
            nc.tensor.matmul(mem_ps[:], tt_sb[:, 1, :], d["u"][:],
                             start=False, stop=True)
            # r = 64*q*mem -- 64x lifts r into fp8 range; the out GEMM's
            # 16x weight scale makes the memory term 1024x, matched by the
            # 1024x identity residual and the 2^-10 descale on the y copy.
            r_sb = p_r.tile([TS, M], bf16, tag="r")
            nc.vector.scalar_tensor_tensor(r_sb[:], d["q"][:], 64.0,
                                           mem_ps[:], ALU.mult, ALU.mult)
            d["r"] = r_sb

        def s6(t):
            """rT PE transpose (cast fp8 on copy)."""
            st[t]["rT"] = pe_transpose(st[t]["r"][:], NM, p_rT, "rT",
                                       dst_dt=fp8)

        def s7(t):
            """out GEMM (+ x residual in-PSUM); y copy; y store."""
            d = st[t]
            out_ps = p_B.tile([TS, D], f32, tag="B", name="out_ps")
            for cb in range(2):
                cols = slice(cb * M, (cb + 1) * M)
                for c in range(2):
                    pr = slice(2 * c, 2 * c + 2)
                    nc.tensor.matmul(out_ps[:, cols], d["rT"][:, pr, :],
                                     wu_sb[:, pr, cols], perf_mode=DR,
                                     start=(c == 0), stop=False)
                # residual: + (1024*I).T @ x_row  (matches the 64*16 scale
                # on the memory term; y copy descales by 2^-10)
                nc.tensor.matmul(out_ps[:, cols], id_sb[:],
                                 d["xr"][:, cols],
                                 start=False, stop=(not has_bu))
                if has_bu:
                    nc.tensor.matmul(out_ps[:, cols], ones_sb[:, 0:1],
                                     opt_sb["bu_r"][:, cols],
                                     start=False, stop=True)
            y_sb = p_y.tile([TS, D], bf16, tag="y")
            nc.scalar.activation(y_sb[:], out_ps[:], AF.Identity,
                                 scale=1.0 / 1024.0)
            s0r = (t - 1) * TS
            nc.sync.dma_start(y[s0r:s0r + TS, :], y_sb[:])
            # drop references so pools can recycle
            del st[t]

        stages = (
            (dma_load, 0, 0), (tx, 1, 0), (s0, 2, 0), (s1, 3, 0),
            (s2, 4, 0), (s3, 5, 0), (s4, 6, 0), (s5, 7, 1), (s6, 8, 1),
            (s7, 9, 1),
        )
        for it in range(NT + 10):
            for fn, lag, tmin in stages:
                t = it - lag
                if tmin <= t <= NT:
                    fn(t)

    _fix_matmult_waits(nc)
    return nc


def _prep_inputs(x, Wd, bd, Wq, bq, Wk, bk, Wv, bv, gq, bq_ln, gk, bk_ln,
                 W1, W2, Wu, bu, adaptive_lr, forget_factor):
    """Host-side: flags, decay matrix, per-core slabs (bf16)."""
    f = np.float32
    bd, bq, bk, bv, bu = (np.asarray(a, f) for a in (bd, bq, bk, bv, bu))
    gq, bq_ln, gk, bk_ln = (np.asarray(a, f) for a in (gq, bq_ln, gk, bk_ln))
    Wd, Wq, Wk, Wv, W1, W2, Wu = (np.asarray(a, f)
                                  for a in (Wd, Wq, Wk, Wv, W1, W2, Wu))
    # fold bd into the k/v/q biases (h = x@Wd + bd only feeds k,v,q)
    if bd.any():
        bk = bk + bd @ Wk
        bv = bv + bd @ Wv
        bq = bq + bd @ Wq
    flags = (bool(bk.any()), bool(bv.any()), bool(bq.any()),
             bool((gq != 1).any()), bool(bq_ln.any()),
             bool((gk != 1).any()), bool(bk_ln.any()), bool(bu.any()))
    (has_bk, has_bv, has_bq, has_gq, has_bqln, has_gk, has_bkln,
     has_bu) = flags

    g = 1.0 / (1.0 + np.exp(-np.float64(forget_factor)))
    lr = np.float64(adaptive_lr)
    t_idx = np.arange(TS)
    lag_cur = t_idx[:, None] - t_idx[None, :]
    Tcur = np.where(lag_cur >= 0, g ** np.maximum(lag_cur, 0), 0.0) * lr
    lag_prev = t_idx[:, None] + TS - t_idx[None, :]
    Tprev = (g ** lag_prev) * lr
    # 1/16 descales the x16 fp8 weight scaling on the v/pred GEMMs
    TT = (np.concatenate([Tprev, Tcur], axis=1).T / 16.0).astype(f)

    def seg(w, nk=None):
        w = np.asarray(w, f)          # [K, N] -> [128, nk, N]
        nk = w.shape[0] // TS
        return np.ascontiguousarray(
            w.reshape(nk, TS, w.shape[1]).transpose(1, 0, 2))

    def to8(a):
        return np.clip(a, -240, 240).astype(F8)

    WS = 16.0  # fp8 weight scale: sigma 0.044 -> 0.7
    wkvq = np.concatenate([Wk, Wv, Wq], axis=1)   # [512, 1536]
    common = {
        "wd": seg(Wd).astype(BF),
        "wkvq": to8(seg(wkvq * WS)),
        "w1": seg(W1).astype(BF),
        "w2": to8(seg(-W2 * WS)),
        "wu": to8(seg(Wu * WS)),
        "tt": seg(TT).astype(BF),
        "ident": (np.eye(TS, dtype=f) * 1024.0).astype(BF),
    }
    if has_bk:
        common["bk_r"] = np.ascontiguousarray(bk[None, :] * WS).astype(BF)
    if has_bv:
        common["bv_r"] = np.ascontiguousarray(bv[None, :] * WS).astype(BF)
    if has_bq:
        common["bq_r"] = np.ascontiguousarray(bq[None, :] * WS).astype(BF)
    if has_bu:
        common["bu_r"] = np.ascontiguousarray(bu[None, :] * 1024.0).astype(BF)
    for name, used, vec in (("gq_b", has_gq, gq), ("bqln_b", has_bqln, bq_ln),
                            ("gk_b", has_gk, gk), ("bkln_b", has_bkln, bk_ln)):
        if used:
            common[name] = np.ascontiguousarray(
                np.broadcast_to(vec, (TS, vec.shape[0])), f)

    x = np.asarray(x, f)
    in_maps = []
    for c in range(N_CORES):
        b, sh = c // 2, c % 2
        if sh == 0:
            haloblk = np.zeros((TS, D), f)
            hm = np.zeros((TS, 1), f)
        else:
            haloblk = x[b, HALF - TS:HALF]
            hm = np.ones((TS, 1), f)
        slab = np.concatenate([haloblk, x[b, sh * HALF:(sh + 1) * HALF]],
                              axis=0)
        m = dict(common)
        m["x_slab"] = np.ascontiguousarray(slab).astype(BF)
        m["hmask"] = hm
        in_maps.append(m)
    return flags, in_maps


def kernel(**inputs):
    global LAST_RESULTS
    flags, in_maps = _prep_inputs(**inputs)
    if flags not in _PROG_CACHE:
        _PROG_CACHE[flags] = _build_program(flags)
    nc = _PROG_CACHE[flags]

    res = run_bass_kernel_spmd(nc, in_maps, list(range(N_CORES)),
                               trace=TRACE, trace_kwargs=TRACE_KWARGS)
    LAST_RESULTS = res

    out = np.empty((B, S, D), np.float32)
    for c in range(N_CORES):
        b, sh = c // 2, c % 2
        out[b, sh * HALF:(sh + 1) * HALF] = np.asarray(
            res.results[c]["y"], dtype=np.float32)
    return out


if __name__ == "__main__":
    print("kernel module for AdvancedNeuralMemory; use test.py to run")
